# revision 1
# baseline (speedup 1.0000x reference)
"""Trainium2 Bass kernel for scatter(w_est -> W[rows, cols]) followed by X @ W.

Strategy (data-parallel over rows, 8 NeuronCores):
  - Host: scatter w_est into W (256x256) - tiny, and numpy assignment matches
    the reference's last-write-wins scatter semantics.
  - Host: shard X row-wise into 8 shards of 62500 rows; transpose each shard
    to feature-major [256, rows] (TensorE contracts over the partition dim)
    and pad rows to 62976 = 123 * 512.
  - Precision/speed: X and W are pre-scaled by 2048, split into fp16 hi/lo
    pairs (hi = fp16(v), lo = fp16(v - hi), ~22 mantissa bits recovered) and
    the product is computed as Xh@Wh + Xl@Wh + Xh@Wl (the ll term is ~2^-34
    and dropped). fp16 streams through the PE at 1 cycle/row vs 4 for fp32,
    so full fp32-class accuracy (~3e-7 rel) costs only 3/4 of one fp32
    matmul. The scaling keeps the lo parts out of fp16-denormal range; the
    descale (2^-22) is folded into the PSUM->SBUF copy (exact).
  - Device (per core): outT[m-chunk, blk] = sum over k-chunk/products of
    W[k,m].T @ XT[k, blk] for 512-row blocks, up to 4 blocks in flight
    filling all 8 PSUM banks; each bank's 6-matmul accumulation finishes
    before the next bank starts (bank-major) so banks recycle early.
    Output is written feature-major so every DMA descriptor moves 8KB
    contiguous, keeping HBM write bandwidth near peak; the host transposes
    each core's outT back and concatenates.
"""

import numpy as np

N_ROWS = 500000
D = 256
N_CORES = 8
RPC = N_ROWS // N_CORES            # 62500 rows per core
BLK = 512                          # rows per matmul (moving free dim)
N_BLK = (RPC + BLK - 1) // BLK     # 123 blocks
RPC_PAD = N_BLK * BLK              # 62976 (0.76% pad)

_CACHE = {}
LAST_RESULT = None  # BassKernelResults of the most recent run (for profiling)

SCALE = 2048.0          # per-operand pre-scale (keeps fp16 lo parts normal)
DESCALE = 1.0 / (SCALE * SCALE)


def _build():
    import concourse.tile as tile
    from concourse import bacc, mybir

    DT = mybir.dt.float16
    nc = bacc.Bacc("TRN2", target_bir_lowering=False, debug=False,
                   num_devices=N_CORES)
    xh = nc.dram_tensor("xh", [D, RPC_PAD], DT, kind="ExternalInput").ap()
    xl = nc.dram_tensor("xl", [D, RPC_PAD], DT, kind="ExternalInput").ap()
    w2 = nc.dram_tensor("w2", [2, D, D], DT, kind="ExternalInput").ap()
    outT = nc.dram_tensor("outT", [D, RPC_PAD], mybir.dt.float32,
                          kind="ExternalOutput").ap()

    with tile.TileContext(nc) as tc:
        with tc.tile_pool(name="wpool", bufs=1) as wpool, \
             tc.tile_pool(name="xpool", bufs=4) as xpool, \
             tc.tile_pool(name="opool", bufs=3) as opool, \
             tc.psum_pool(name="pspool", bufs=1) as pspool:
            # wt[h][k][m] = Whl[h][k*128:(k+1)*128, m*128:(m+1)*128]
            # W loads go on the scalar HWDGE ring so they don't delay the
            # first X chunk on the sync ring.
            wt = [[[None, None], [None, None]] for _ in range(2)]
            for h in range(2):
                for k in range(2):
                    for m in range(2):
                        t = wpool.tile([128, 128], DT, name=f"w{h}{k}{m}",
                                       tag=f"w{h}{k}{m}")
                        nc.scalar.dma_start(
                            t[:], w2[h, k * 128:(k + 1) * 128,
                                     m * 128:(m + 1) * 128])
                        wt[h][k][m] = t

            # (wh,xh), (wh,xl), (wl,xh) per k-chunk; 6 matmuls per PSUM bank
            prods = [(0, 0), (0, 1), (1, 0)]
            steps = [(k, wh_, xh_) for k in range(2) for (wh_, xh_) in prods]

            b0 = 0
            while b0 < N_BLK:
                # blocks per input chunk: small first chunk so the PE
                # starts early, then 8 blocks (= 1 MiB per DMA stream)
                cb = 4 if b0 == 0 else min(8, N_BLK - b0)
                c0 = b0 * BLK
                x = [[None, None], [None, None]]  # x[h][k]
                for h, src in ((0, xh), (1, xl)):
                    for k in range(2):
                        t = xpool.tile([128, cb * BLK], DT, name=f"x{h}{k}",
                                       tag=f"x{h}{k}")
                        nc.sync.dma_start(
                            t[:], src[k * 128:(k + 1) * 128,
                                      c0:c0 + cb * BLK])
                        x[h][k] = t

                gi = 0
                while gi < cb:
                    gb = min(4, cb - gi)       # blocks in this PSUM group
                    gc0 = c0 + gi * BLK
                    for m in range(2):
                        st = opool.tile([128, gb * BLK], mybir.dt.float32,
                                        name=f"st{m}", tag=f"st{m}")
                        for b in range(gb):
                            # bank-major: finish one PSUM bank's whole
                            # 6-matmul accumulation, copy it out, move on -
                            # banks free early and DVE work stays spread.
                            ps = pspool.tile([128, BLK], mybir.dt.float32,
                                             name=f"ps{m}{b}",
                                             tag=f"ps{m}{b}")
                            sl = slice((gi + b) * BLK, (gi + b + 1) * BLK)
                            for si, (k, hw_, hx_) in enumerate(steps):
                                nc.tensor.matmul(
                                    ps[:], wt[hw_][k][m][:],
                                    x[hx_][k][:, sl],
                                    start=(si == 0),
                                    stop=(si == len(steps) - 1))
                            nc.vector.tensor_scalar_mul(
                                st[:, b * BLK:(b + 1) * BLK], ps[:],
                                DESCALE)
                        nc.scalar.dma_start(
                            outT[m * 128:(m + 1) * 128,
                                 gc0:gc0 + gb * BLK], st[:])
                    gi += gb
                b0 += cb

    nc.compile()
    return nc


def kernel(X, w_est, rows, cols):
    global LAST_RESULT
    from concourse.bass_utils import run_bass_kernel_spmd

    X = np.asarray(X, dtype=np.float32)
    w_est = np.asarray(w_est, dtype=np.float32)
    rows = np.asarray(rows)
    cols = np.asarray(cols)

    W = np.zeros((D, D), dtype=np.float32)
    W[rows, cols] = w_est  # last-write-wins, same as XLA scatter-set

    if "nc" not in _CACHE:
        _CACHE["nc"] = _build()
    nc = _CACHE["nc"]

    Ws = W * SCALE
    Wh = Ws.astype(np.float16)
    Wl = (Ws - Wh.astype(np.float32)).astype(np.float16)
    w2 = np.stack([Wh, Wl])

    in_maps = []
    for c in range(N_CORES):
        shard = X[c * RPC:(c + 1) * RPC].T * SCALE   # [256, 62500] fp32
        xh = np.zeros((D, RPC_PAD), dtype=np.float16)
        xh[:, :RPC] = shard.astype(np.float16)
        xl = np.zeros((D, RPC_PAD), dtype=np.float16)
        xl[:, :RPC] = (shard - xh[:, :RPC].astype(np.float32)
                       ).astype(np.float16)
        in_maps.append({"xh": xh, "xl": xl, "w2": w2})

    # the axon-tunneled device occasionally reports a transient
    # NRT_EXEC_UNIT_UNRECOVERABLE on the first run after another process
    # used it; a retry recovers.
    last_exc = None
    for attempt in range(3):
        try:
            res = run_bass_kernel_spmd(nc, in_maps,
                                       core_ids=list(range(N_CORES)))
            break
        except Exception as e:
            last_exc = e
            import time
            time.sleep(10.0 * (attempt + 1))
    else:
        raise last_exc
    LAST_RESULT = res
    return np.concatenate(
        [np.ascontiguousarray(r["outT"][:, :RPC].T) for r in res.results],
        axis=0)



# revision 2
# speedup vs baseline: 1.8031x; 1.8031x over previous
"""Trainium2 Bass kernel for scatter(w_est -> W[rows, cols]) followed by X @ W.

Strategy (data-parallel over rows, 8 NeuronCores):
  - Host: scatter w_est into W (256x256) - tiny; numpy assignment matches the
    reference's last-write-wins scatter semantics.
  - Host: shard X row-wise into 8 shards of 62500 rows; transpose each shard
    to feature-major [256, rows] (TensorE contracts over the partition dim)
    and pad rows to 62976 = 123 * 512.
  - Precision/speed: the correctness gate is rel_err < 2e-2, so a single
    fp16 matmul (rel err ~5e-4) is plenty: X and W are cast to fp16, the
    product accumulates in fp32 PSUM, and the output is stored as fp16
    (host upcasts to fp32). This halves both input and output HBM traffic
    vs the fp32-accurate hi/lo scheme and cuts PE work 3x; the kernel is
    then HBM-bound at ~64.5 MB / 358 GB/s ~ 180 us per core.
  - Device (per core): outT[m-chunk, blk] = sum over k-chunk of
    W[k,m].T @ XT[k, blk] for 512-row blocks; 4 blocks per PSUM group so
    all 8 banks stay in flight; bank-major accumulation so banks recycle
    early. Input streams on the sync HWDGE ring, output on the scalar
    ring — both rings carry ~32 MB so the two together saturate the
    per-core HBM budget.
"""

import numpy as np

N_ROWS = 500000
D = 256
N_CORES = 8
RPC = N_ROWS // N_CORES            # 62500 rows per core
BLK = 512                          # rows per matmul (moving free dim)
N_BLK = (RPC + BLK - 1) // BLK     # 123 blocks
RPC_PAD = N_BLK * BLK              # 62976 (0.76% pad)

_CACHE = {}
LAST_RESULT = None  # BassKernelResults of the most recent run (for profiling)


def _build():
    import concourse.tile as tile
    from concourse import bacc, mybir

    DT = mybir.dt.float16
    nc = bacc.Bacc("TRN2", target_bir_lowering=False, debug=False,
                   num_devices=N_CORES)
    xh = nc.dram_tensor("xh", [D, RPC_PAD], DT, kind="ExternalInput").ap()
    w = nc.dram_tensor("w", [D, D], DT, kind="ExternalInput").ap()
    outT = nc.dram_tensor("outT", [D, RPC_PAD], DT, kind="ExternalOutput").ap()

    with tile.TileContext(nc) as tc:
        with tc.tile_pool(name="wpool", bufs=1) as wpool, \
             tc.tile_pool(name="xpool", bufs=4) as xpool, \
             tc.tile_pool(name="opool", bufs=3) as opool, \
             tc.psum_pool(name="pspool", bufs=1) as pspool:
            # wt[k][m] = W[k*128:(k+1)*128, m*128:(m+1)*128]; W loads go on
            # the scalar HWDGE ring so they don't delay the first X chunk.
            wt = [[None, None], [None, None]]
            for k in range(2):
                for m in range(2):
                    t = wpool.tile([128, 128], DT, name=f"w{k}{m}",
                                   tag=f"w{k}{m}")
                    nc.scalar.dma_start(
                        t[:], w[k * 128:(k + 1) * 128,
                                m * 128:(m + 1) * 128])
                    wt[k][m] = t

            b0 = 0
            while b0 < N_BLK:
                # blocks per input chunk: small first chunk so the PE
                # starts early, then 8 blocks (= 1 MiB per DMA stream)
                cb = 4 if b0 == 0 else min(8, N_BLK - b0)
                c0 = b0 * BLK
                x = [None, None]  # x[k]
                for k in range(2):
                    t = xpool.tile([128, cb * BLK], DT, name=f"x{k}",
                                   tag=f"x{k}")
                    nc.sync.dma_start(
                        t[:], xh[k * 128:(k + 1) * 128, c0:c0 + cb * BLK])
                    x[k] = t

                gi = 0
                while gi < cb:
                    gb = min(4, cb - gi)       # blocks in this PSUM group
                    gc0 = c0 + gi * BLK
                    for m in range(2):
                        st = opool.tile([128, gb * BLK], DT,
                                        name=f"st{m}", tag=f"st{m}")
                        for b in range(gb):
                            # bank-major: finish one PSUM bank's 2-matmul
                            # accumulation, copy it out, move on - banks
                            # free early and DVE work stays spread.
                            ps = pspool.tile([128, BLK], mybir.dt.float32,
                                             name=f"ps{m}{b}",
                                             tag=f"ps{m}{b}")
                            sl = slice((gi + b) * BLK, (gi + b + 1) * BLK)
                            for k in range(2):
                                nc.tensor.matmul(
                                    ps[:], wt[k][m][:], x[k][:, sl],
                                    start=(k == 0), stop=(k == 1))
                            nc.vector.tensor_scalar_mul(
                                st[:, b * BLK:(b + 1) * BLK], ps[:], 1.0)
                        nc.scalar.dma_start(
                            outT[m * 128:(m + 1) * 128,
                                 gc0:gc0 + gb * BLK], st[:])
                    gi += gb
                b0 += cb

    nc.compile()
    return nc


def kernel(X, w_est, rows, cols):
    global LAST_RESULT
    from concourse.bass_utils import run_bass_kernel_spmd

    X = np.asarray(X, dtype=np.float32)
    w_est = np.asarray(w_est, dtype=np.float32)
    rows = np.asarray(rows)
    cols = np.asarray(cols)

    W = np.zeros((D, D), dtype=np.float32)
    W[rows, cols] = w_est  # last-write-wins, same as XLA scatter-set

    if "nc" not in _CACHE:
        _CACHE["nc"] = _build()
    nc = _CACHE["nc"]

    w16 = W.astype(np.float16)
    in_maps = []
    for c in range(N_CORES):
        shard = X[c * RPC:(c + 1) * RPC].T   # [256, 62500] fp32
        xh = np.zeros((D, RPC_PAD), dtype=np.float16)
        xh[:, :RPC] = shard.astype(np.float16)
        in_maps.append({"xh": xh, "w": w16})

    # the axon-tunneled device occasionally reports a transient
    # NRT_EXEC_UNIT_UNRECOVERABLE on the first run after another process
    # used it; a retry recovers.
    last_exc = None
    for attempt in range(3):
        try:
            res = run_bass_kernel_spmd(nc, in_maps,
                                       core_ids=list(range(N_CORES)))
            break
        except Exception as e:
            last_exc = e
            import time
            time.sleep(10.0 * (attempt + 1))
    else:
        raise last_exc
    LAST_RESULT = res
    return np.concatenate(
        [np.ascontiguousarray(r["outT"][:, :RPC].T).astype(np.float32)
         for r in res.results],
        axis=0)
